# revision 1
# baseline (speedup 1.0000x reference)
"""Masked transformer encoder layer on 8 trn2 NeuronCores.

Sharding: pure data-parallel — batch B=8, one batch element per core, zero
collectives.  Each core runs the full layer on (N=1024, D=1024, H=16, F=4096).

Per-core pipeline (bf16 matmuls, fp32 accumulation / layernorm math):
  LN0 (token-major, bn_stats) -> h bf16 -> PE-transpose -> h^T (feature-major)
  q^T,k^T = Wqkv(q,k) @ h^T   (feature-major out)
  v       = h^T.T @ Wqkv(v)   (token-major out, +ones column for denominators)
  S^T     = k^T.T @ q^T   per head   (K=64, auto row-group packing)
  P^T     = exp(0.125*S^T + key_mask_bias)   (ACT, per-partition bias)
  out^T   = v_aug.T @ P^T  (row 64 = softmax denominator)
  attn^T  = out^T * bcast(1/denom)
  A       = attn^T.T @ Wproj^T (token-major) ; x1 = src + (1-mq)*w + mq*(A+bproj)
            (w = Wproj @ mean_j(v) + bproj handles fully-masked query rows)
  LN1 in-place (x1 -> x1n fp32) -> transpose -> x1n^T
  z^T     = W1 @ x1n^T ; gelu(+b1) ; y^T = W2 @ z^T
  out     = x1n + y^T.T + b2
"""

import numpy as np
import ml_dtypes

import concourse.bass as bass
import concourse.tile as tile
from concourse import bacc
from concourse import mybir
from concourse.bass_utils import run_bass_kernel_spmd

B, N, D, H, F = 8, 1024, 1024, 16, 4096
HD = D // H          # 64
P = 128
FC = D // P          # 8 feature chunks of D
TT = N // P          # 8 token tiles
GC = F // P          # 32 chunks of F
NEG = -1e30
EPS = 1e-5

f32 = mybir.dt.float32
bf16 = mybir.dt.bfloat16
AF = mybir.ActivationFunctionType
OP = mybir.AluOpType


def _layernorm_inplace_stats(nc, pools, x_ap):
    """Return (mean, rstd) APs ([128,1] each) for x_ap [128, 1024] fp32."""
    stats = pools["stats"].tile([P, 2, 6], f32)
    for sg in range(2):
        nc.vector.bn_stats(out=stats[:, sg, :], in_=x_ap[:, sg * 512:(sg + 1) * 512])
    mv = pools["mv"].tile([P, 2], f32)
    nc.vector.bn_aggr(out=mv[:], in_=stats[:])
    # rstd = 1/sqrt(var + eps)
    nc.scalar.activation(out=mv[:, 1:2], in_=mv[:, 1:2], func=AF.Sqrt,
                         bias=pools["eps"][:], scale=1.0)
    nc.vector.reciprocal(out=mv[:, 1:2], in_=mv[:, 1:2])
    return mv[:, 0:1], mv[:, 1:2]


def build_bass():
    nc = bacc.Bacc("TRN2")

    # ---------------- DRAM I/O ----------------
    src_h = nc.dram_tensor("src", [N, D], f32, kind="ExternalInput")
    kb_h = nc.dram_tensor("kbias", [TT, P], f32, kind="ExternalInput")
    mq_h = nc.dram_tensor("mq", [TT, P], f32, kind="ExternalInput")
    vecs_h = nc.dram_tensor("vecs", [6, D], f32, kind="ExternalInput")
    b1_h = nc.dram_tensor("b1r", [GC, P], f32, kind="ExternalInput")
    wqkv_h = nc.dram_tensor("wqkvT", [FC, P, 3 * D], bf16, kind="ExternalInput")
    wproj_h = nc.dram_tensor("wprojT", [FC, P, D], bf16, kind="ExternalInput")
    w1_h = nc.dram_tensor("w1T", [FC, P, F], bf16, kind="ExternalInput")
    w2_h = nc.dram_tensor("w2T", [GC, P, D], bf16, kind="ExternalInput")
    out_h = nc.dram_tensor("out", [N, D], f32, kind="ExternalOutput")

    with TileKernel(nc) as tk:
        tk.run(src_h, kb_h, mq_h, vecs_h, b1_h, wqkv_h, wproj_h, w1_h, w2_h, out_h)
    nc.compile()
    return nc


class TileKernel:
    def __init__(self, nc):
        self.nc = nc
        self.tc = tile.TileContext(nc)

    def __enter__(self):
        self.tc.__enter__()
        return self

    def __exit__(self, *a):
        return self.tc.__exit__(*a)

    def run(self, src_h, kb_h, mq_h, vecs_h, b1_h, wqkv_h, wproj_h, w1_h, w2_h, out_h):
        nc, tc = self.nc, self.tc
        from contextlib import ExitStack

        with ExitStack() as top:
            consts = top.enter_context(tc.tile_pool(name="consts", bufs=1))
            persist = top.enter_context(tc.tile_pool(name="persist", bufs=1))
            tmp_pool = top.enter_context(tc.tile_pool(name="tmp", bufs=2))
            stats_pool = top.enter_context(tc.tile_pool(name="stats", bufs=3))
            mv_pool = top.enter_context(tc.tile_pool(name="mv", bufs=4))
            
            # ---------- constants ----------
            ident = consts.tile([P, P], bf16)
            from concourse.masks import make_identity
            make_identity(nc, ident[:])
            ones_row = consts.tile([1, P], f32)
            nc.vector.memset(ones_row[:], 1.0)
            ones_col = consts.tile([P, 1], bf16)
            nc.vector.memset(ones_col[:], 1.0)
            ones_row_bf = consts.tile([1, P], bf16)
            nc.vector.memset(ones_row_bf[:], 1.0)
            eps_sb = consts.tile([P, 1], f32)
            nc.vector.memset(eps_sb[:], EPS)
            pools = {"stats": stats_pool, "mv": mv_pool, "eps": eps_sb}

            # DMA order tuned for startup: g0/beta0 broadcasts, then src
            # (LN0 gates everything), then the rest of the small constants.
            bcast = consts.tile([P, 6, D], f32)

            def _bcast_dma(v6):
                bc_src = bass.AP(tensor=vecs_h[0:1, :].tensor, offset=v6 * D,
                                 ap=[[0, P], [1, D]])
                nc.sync.dma_start(out=bcast[:, v6, :], in_=bc_src)

            for v6 in (0, 1):
                _bcast_dma(v6)

            src_sb = persist.tile([P, TT, D], f32)   # src -> srcw -> x1 -> x1n
            for tt in range(TT):
                nc.sync.dma_start(out=src_sb[:, tt, :],
                                  in_=src_h[tt * P:(tt + 1) * P, :])

            kb_sb = consts.tile([P, TT], f32)
            nc.sync.dma_start(out=kb_sb[:], in_=kb_h[:, :].rearrange("a p -> p a"))
            mq_sb = consts.tile([P, TT], f32)
            nc.sync.dma_start(out=mq_sb[:], in_=mq_h[:, :].rearrange("a p -> p a"))
            invmq_sb = consts.tile([P, TT], f32)
            nc.vector.tensor_scalar(out=invmq_sb[:], in0=mq_sb[:], scalar1=-1.0,
                                    scalar2=1.0, op0=OP.mult, op1=OP.add)
            b1_sb = consts.tile([P, GC], f32)
            nc.sync.dma_start(out=b1_sb[:], in_=b1_h[:, :].rearrange("g p -> p g"))
            for v6 in (2, 3, 4, 5):
                _bcast_dma(v6)
            g0b, beta0b = bcast[:, 0], bcast[:, 1]
            g1b, beta1b = bcast[:, 2], bcast[:, 3]
            bprojb, b2b = bcast[:, 4], bcast[:, 5]

            wb_sb = consts.tile([P, D], f32)       # (Wproj @ mean_j v + bproj) broadcast
            u_sb = consts.tile([P, FC], bf16)      # mean_j v, feature-major columns

            with ExitStack() as attn_scope:
                qkT = attn_scope.enter_context(tc.tile_pool(name="qkT", bufs=1))
                vp = attn_scope.enter_context(tc.tile_pool(name="vsb", bufs=1))

                qkT_sb = qkT.tile([P, 16, N], bf16)
                v_sb = vp.tile([P, TT, H, HD + 1], bf16)
                nc.vector.memset(v_sb[:, :, :, HD:HD + 1], 1.0)

                # ================= LN0 + transpose + QKV =================
                with ExitStack() as qkv_scope:
                    htp = qkv_scope.enter_context(tc.tile_pool(name="hT", bufs=1))
                    hbp = qkv_scope.enter_context(tc.tile_pool(name="hbf", bufs=2))
                    trps = qkv_scope.enter_context(
                        tc.tile_pool(name="trps", bufs=2, space="PSUM"))
                    qkps = qkv_scope.enter_context(
                        tc.tile_pool(name="qkps", bufs=3, space="PSUM"))

                    hT_sb = htp.tile([P, FC, N], bf16)

                    for tt in range(TT):
                        x = src_sb[:, tt, :]
                        mean, rstd = _layernorm_inplace_stats(nc, pools, x)
                        ht = tmp_pool.tile([P, D], f32, tag="lnt")
                        nc.vector.tensor_scalar(out=ht[:], in0=x, scalar1=mean,
                                                scalar2=rstd, op0=OP.subtract, op1=OP.mult)
                        nc.gpsimd.tensor_tensor(ht[:], ht[:], g0b, OP.mult)
                        hbf = hbp.tile([P, D], bf16)
                        nc.vector.tensor_tensor(hbf[:], ht[:], beta0b, OP.add)
                        for fb in range(FC):
                            ps = trps.tile([P, P], bf16)
                            nc.tensor.transpose(ps[:], hbf[:, fb * P:(fb + 1) * P], ident[:])
                            nc.scalar.copy(hT_sb[:, fb, tt * P:(tt + 1) * P], ps[:])

                    # q^T, k^T (feature-major)
                    with tc.tile_pool(name="wqk", bufs=1) as wqkp:
                        wqk_sb = wqkp.tile([P, FC, 2 * D], bf16)
                        for fc in range(FC):
                            nc.sync.dma_start(out=wqk_sb[:, fc, :],
                                              in_=wqkv_h[fc, :, 0:2 * D])
                        for oc in range(16):
                            ps = qkps.tile([P, 1024], f32)
                            for ib in range(2):
                                for fc in range(FC):
                                    nc.tensor.matmul(
                                        ps[:, ib * 512:(ib + 1) * 512],
                                        wqk_sb[:, fc, oc * P:(oc + 1) * P],
                                        hT_sb[:, fc, ib * 512:(ib + 1) * 512],
                                        start=(fc == 0), stop=(fc == FC - 1))
                            nc.vector.tensor_copy(qkT_sb[:, oc, :], ps[:])

                    # v (token-major) into per-head lhsT layout
                    with tc.tile_pool(name="wv", bufs=1) as wvp:
                        wv_sb = wvp.tile([P, FC, D], bf16)
                        nc.sync.dma_start(out=wv_sb[:],
                                          in_=wqkv_h[:, :, 2 * D:3 * D].rearrange("f p o -> p f o"))
                        for tt in range(TT):
                            ps = qkps.tile([P, 1024], f32)
                            for vb in range(2):
                                for fc in range(FC):
                                    nc.tensor.matmul(
                                        ps[:, vb * 512:(vb + 1) * 512],
                                        hT_sb[:, fc, tt * P:(tt + 1) * P],
                                        wv_sb[:, fc, vb * 512:(vb + 1) * 512],
                                        start=(fc == 0), stop=(fc == FC - 1))
                            nc.vector.tensor_copy(
                                v_sb[:, tt, :, 0:HD],
                                ps[:].rearrange("p (h c) -> p h c", h=H))

                atp = attn_scope.enter_context(tc.tile_pool(name="attnT", bufs=1))
                wpp = attn_scope.enter_context(tc.tile_pool(name="wproj", bufs=1))
                attnT_sb = atp.tile([P, FC, N], bf16)
                wproj_sb = wpp.tile([P, FC, D], bf16)
                nc.sync.dma_start(out=wproj_sb[:],
                                  in_=wproj_h[:, :, :].rearrange("f p o -> p f o"))

                # ============ u = mean_j v ; w = Wproj @ u + bproj ============
                with tc.tile_pool(name="uwps", bufs=2, space="PSUM") as uwps, \
                        tc.tile_pool(name="wrowp", bufs=1) as wrowp:
                    wrow = wrowp.tile([1, D], f32)
                    for fc in range(FC):
                        ps = uwps.tile([P, 512], f32, tag="ups")
                        for hh in range(2):
                            for jc in range(TT):
                                nc.tensor.matmul(ps[hh * HD:(hh + 1) * HD, 0:1],
                                                 v_sb[:, jc, 2 * fc + hh, 0:HD],
                                                 ones_col[:],
                                                 start=(jc == 0), stop=(jc == TT - 1))
                        nc.vector.tensor_scalar(out=u_sb[:, fc:fc + 1], in0=ps[:, 0:1],
                                                scalar1=1.0 / N, scalar2=None, op0=OP.mult)
                    for ob in range(2):
                        ps = uwps.tile([P, 512], f32, tag="wps")
                        for fc in range(FC):
                            nc.tensor.matmul(ps[0:1, :], u_sb[:, fc:fc + 1],
                                             wproj_sb[:, fc, ob * 512:(ob + 1) * 512],
                                             start=(fc == 0), stop=(fc == FC - 1))
                        nc.vector.tensor_tensor(wrow[:, ob * 512:(ob + 1) * 512], ps[0:1, :],
                                                bprojb[0:1, ob * 512:(ob + 1) * 512], OP.add)
                    for ob in range(2):
                        ps = uwps.tile([P, 512], f32, tag="wbps")
                        nc.tensor.matmul(ps[:], ones_row[:],
                                         wrow[:, ob * 512:(ob + 1) * 512],
                                         start=True, stop=True)
                        nc.vector.tensor_copy(wb_sb[:, ob * 512:(ob + 1) * 512], ps[:])

                # srcw = src + (1-mq)*wb + mq*bprojb   (in place)
                for tt in range(TT):
                    t = tmp_pool.tile([P, D], f32, tag="srcw")
                    nc.vector.tensor_scalar(out=t[:], in0=wb_sb[:],
                                            scalar1=invmq_sb[:, tt:tt + 1],
                                            scalar2=None, op0=OP.mult)
                    nc.vector.tensor_tensor(src_sb[:, tt, :], src_sb[:, tt, :], t[:], OP.add)
                    t2 = tmp_pool.tile([P, D], f32, tag="srcw")
                    nc.gpsimd.tensor_scalar(out=t2[:], in0=bprojb[:],
                                            scalar1=mq_sb[:, tt:tt + 1],
                                            scalar2=None, op0=OP.mult)
                    nc.gpsimd.tensor_tensor(src_sb[:, tt, :], src_sb[:, tt, :], t2[:], OP.add)

                # ================= attention + proj =================
                with ExitStack() as att:
                    ptp = att.enter_context(tc.tile_pool(name="pt", bufs=12))
                    rdp = att.enter_context(tc.tile_pool(name="rd", bufs=3))
                    dnp = att.enter_context(tc.tile_pool(name="dn", bufs=3))
                    sps = att.enter_context(tc.tile_pool(name="sps", bufs=2, space="PSUM"))
                    avps = att.enter_context(tc.tile_pool(name="avps", bufs=3, space="PSUM"))
                    bcps2 = att.enter_context(tc.tile_pool(name="bcps2", bufs=1, space="PSUM"))

                    for h in range(H):
                        hp = (h % 2) * HD
                        fc_h = h // 2
                        # S^T for both i-blocks into one 2-bank psum; single exp
                        pts = []
                        for jc in range(TT):
                            ps_s = sps.tile([P, 1024], f32)
                            for ib in range(2):
                                nc.tensor.matmul(
                                    ps_s[:, ib * 512:(ib + 1) * 512],
                                    qkT_sb[hp:hp + HD, 8 + fc_h, jc * P:(jc + 1) * P],
                                    qkT_sb[hp:hp + HD, fc_h, ib * 512:(ib + 1) * 512],
                                    start=True, stop=True)
                            pt = ptp.tile([P, 1024], bf16)
                            nc.scalar.activation(out=pt[:], in_=ps_s[:], func=AF.Exp,
                                                 bias=kb_sb[:, jc:jc + 1], scale=0.125)
                            pts.append(pt)
                        for ib in range(2):
                            isl = slice(ib * 512, (ib + 1) * 512)
                            ps_av = avps.tile([P, 512], f32)
                            for jc in range(TT):
                                nc.tensor.matmul(ps_av[0:HD + 1, :],
                                                 v_sb[:, jc, h, :], pts[jc][:, isl],
                                                 start=(jc == 0), stop=(jc == TT - 1))
                            dn = dnp.tile([1, 512], bf16)
                            nc.vector.tensor_copy(dn[:], ps_av[HD:HD + 1, :])
                            ps_b = bcps2.tile([P, 512], f32)
                            nc.tensor.matmul(ps_b[:], ones_row_bf[:], dn[:],
                                             start=True, stop=True)
                            rd = rdp.tile([P, 512], f32)
                            nc.vector.reciprocal(rd[:], ps_b[:])
                            nc.vector.tensor_tensor(
                                attnT_sb[hp:hp + HD, fc_h, isl],
                                ps_av[0:HD, :], rd[0:HD, :], OP.mult)



                # proj + x1 (into src_sb); own psum scope after attention frees banks
                with tc.tile_pool(name="pps", bufs=3, space="PSUM") as pps:
                    for tt in range(TT):
                        for ob in range(2):
                            osl = slice(ob * 512, (ob + 1) * 512)
                            ps_p = pps.tile([P, 512], f32)
                            for fc in range(FC):
                                nc.tensor.matmul(ps_p[:],
                                                 attnT_sb[:, fc, tt * P:(tt + 1) * P],
                                                 wproj_sb[:, fc, osl],
                                                 start=(fc == 0), stop=(fc == FC - 1))
                            t = tmp_pool.tile([P, 512], f32, tag="x1t")
                            nc.vector.tensor_scalar(out=t[:], in0=ps_p[:],
                                                    scalar1=mq_sb[:, tt:tt + 1],
                                                    scalar2=None, op0=OP.mult)
                            nc.vector.tensor_tensor(src_sb[:, tt, osl],
                                                    src_sb[:, tt, osl], t[:], OP.add)

            # ================= LN1 (in place) + transpose =================
            with ExitStack() as ffn1:
                ztp = ffn1.enter_context(tc.tile_pool(name="zT", bufs=1))
                zT_sb = ztp.tile([P, GC, N], bf16)
                f1 = ffn1.enter_context(ExitStack())
                xtp = f1.enter_context(tc.tile_pool(name="x1nT", bufs=1))
                xbp = f1.enter_context(tc.tile_pool(name="x1nbf", bufs=2))
                w1p = f1.enter_context(tc.tile_pool(name="w1p", bufs=3))
                trps2 = f1.enter_context(tc.tile_pool(name="trps2", bufs=3, space="PSUM"))
                zps = f1.enter_context(tc.tile_pool(name="zps", bufs=2, space="PSUM"))

                x1nT_sb = xtp.tile([P, FC, N], bf16)

                for tt in range(TT):
                    x = src_sb[:, tt, :]
                    mean, rstd = _layernorm_inplace_stats(nc, pools, x)
                    nc.vector.tensor_scalar(out=x, in0=x, scalar1=mean,
                                            scalar2=rstd, op0=OP.subtract, op1=OP.mult)
                    nc.gpsimd.tensor_tensor(x, x, g1b, OP.mult)
                    nc.vector.tensor_tensor(x, x, beta1b, OP.add)
                    xbf = xbp.tile([P, D], bf16)
                    nc.gpsimd.tensor_copy(xbf[:], x)
                    for fb in range(FC):
                        ps = trps2.tile([P, P], bf16)
                        nc.tensor.transpose(ps[:], xbf[:, fb * P:(fb + 1) * P], ident[:])
                        nc.scalar.copy(x1nT_sb[:, fb, tt * P:(tt + 1) * P], ps[:])

                # ---------------- FFN linear1 + gelu ----------------
                for gc in range(GC):
                    w1t = w1p.tile([P, FC, P], bf16)
                    nc.sync.dma_start(out=w1t[:],
                                      in_=w1_h[:, :, gc * P:(gc + 1) * P].rearrange("f p c -> p f c"))
                    ps = zps.tile([P, 1024], f32)
                    for ib in range(2):
                        for fc in range(FC):
                            nc.tensor.matmul(ps[:, ib * 512:(ib + 1) * 512],
                                             w1t[:, fc, :],
                                             x1nT_sb[:, fc, ib * 512:(ib + 1) * 512],
                                             start=(fc == 0), stop=(fc == FC - 1))
                    nc.scalar.activation(out=zT_sb[:, gc, :], in_=ps[:], func=AF.Gelu,
                                         bias=b1_sb[:, gc:gc + 1], scale=1.0)

                # -------- FFN linear2, token-major y, fused residual+out --------
                f1.close()
                with ExitStack() as ffn2:
                    w2p = ffn2.enter_context(tc.tile_pool(name="w2p", bufs=2))
                    yout = ffn2.enter_context(tc.tile_pool(name="yout", bufs=4))
                    yps = ffn2.enter_context(tc.tile_pool(name="yps", bufs=3, space="PSUM"))

                    QW = 256
                    for ob in range(4):
                        osl = slice(ob * QW, (ob + 1) * QW)
                        w2q = w2p.tile([P, GC, QW], bf16)
                        nc.sync.dma_start(out=w2q[:],
                                          in_=w2_h[:, :, osl].rearrange("g p c -> p g c"))
                        for tt in range(TT):
                            ps = yps.tile([P, QW], f32)
                            for gc in range(GC):
                                nc.tensor.matmul(ps[:],
                                                 zT_sb[:, gc, tt * P:(tt + 1) * P],
                                                 w2q[:, gc, :],
                                                 start=(gc == 0), stop=(gc == GC - 1))
                            t = yout.tile([P, QW], f32)
                            nc.vector.tensor_tensor(t[:], ps[:], b2b[:, osl], OP.add)
                            nc.vector.tensor_tensor(t[:], t[:], src_sb[:, tt, osl], OP.add)
                            nc.sync.dma_start(out=out_h[tt * P:(tt + 1) * P, osl], in_=t[:])


_NC_CACHE = {}


def _get_nc():
    if "nc" not in _NC_CACHE:
        _NC_CACHE["nc"] = build_bass()
    return _NC_CACHE["nc"]


def prep_in_maps(inputs):
    src = np.asarray(inputs["src"], dtype=np.float32)          # [B, N, D]
    mask = np.asarray(inputs["mask"])                          # [B, N] bool
    Wqkv = np.asarray(inputs["Wqkv"], dtype=np.float32)
    Wproj = np.asarray(inputs["Wproj"], dtype=np.float32)
    bproj = np.asarray(inputs["bproj"], dtype=np.float32)
    W1 = np.asarray(inputs["W1"], dtype=np.float32)
    b1 = np.asarray(inputs["b1"], dtype=np.float32)
    W2 = np.asarray(inputs["W2"], dtype=np.float32)
    b2 = np.asarray(inputs["b2"], dtype=np.float32)
    g0 = np.asarray(inputs["g0"], dtype=np.float32)
    beta0 = np.asarray(inputs["beta0"], dtype=np.float32)
    g1 = np.asarray(inputs["g1"], dtype=np.float32)
    beta1 = np.asarray(inputs["beta1"], dtype=np.float32)

    bf = ml_dtypes.bfloat16
    wqkvT = np.ascontiguousarray(Wqkv.T).reshape(FC, P, 3 * D).astype(bf)
    wprojT = np.ascontiguousarray(Wproj.T).reshape(FC, P, D).astype(bf)
    w1T = np.ascontiguousarray(W1.T).reshape(FC, P, F).astype(bf)
    w2T = np.ascontiguousarray(W2.T).reshape(GC, P, D).astype(bf)
    vecs = np.ascontiguousarray(np.stack([g0, beta0, g1, beta1, bproj, b2]))
    b1r = np.ascontiguousarray(b1.reshape(GC, P))
    kbias = np.where(mask, 0.0, NEG).astype(np.float32).reshape(B, TT, P)
    mqf = mask.astype(np.float32).reshape(B, TT, P)

    in_maps = []
    for b in range(B):
        in_maps.append({
            "src": np.ascontiguousarray(src[b]),
            "kbias": np.ascontiguousarray(kbias[b]),
            "mq": np.ascontiguousarray(mqf[b]),
            "vecs": vecs,
            "b1r": b1r,
            "wqkvT": wqkvT,
            "wprojT": wprojT,
            "w1T": w1T,
            "w2T": w2T,
        })
    return in_maps


def kernel(**inputs):
    in_maps = prep_in_maps(inputs)
    nc = _get_nc()
    res = run_bass_kernel_spmd(nc, in_maps, core_ids=list(range(B)))
    return np.stack([r["out"] for r in res.results]).astype(np.float32)



# revision 55
# speedup vs baseline: 1.5303x; 1.5303x over previous
"""Masked transformer encoder layer on 8 trn2 NeuronCores.

Sharding: pure data-parallel - batch B=8, one batch element per core, zero
collectives. Each core runs the full layer on (N=1024, D=1024, H=16, F=4096).

Per-core pipeline (fp8e4 DoubleRow matmuls where possible, fp32 psum):
  LN0 (bn_stats; ts -> bf16, tt*g0, tt+beta0) -> h bf16 -> PE-transpose ->
    hT fp8 (psum->sbuf pair copies)
  v   = hT.T @ Wv8 (DR)   -> v_sb fp8 (scale 1/sw folded in copy), +ones col
  per head-pair hp: qT,kT = Wq8/Wk8 @ hT (DR) -> qkT bf16 (carry sw scale)
    per head: S^T = kT.T qT (bf16, K=64); P^T = exp(0.125/sw^2 S^T + kbias)
      -> pt fp8 [128,2,N] jc-pairs (ACT)
    AV: out^T = v_aug.T @ P^T (DR, row 64 = denom); rd = recip(denom) bf16;
    rb = ones x rd (PE bcast); attnT = ps_av * rb -> fp8 (DVE)
  u = mean_j v (PE) ; w = u @ Wproj8/sp + bproj ; wb = bcast(w)
  srcw = src + (1-mq)*wb + mq*bproj (Pool stt x2, in place)
  proj: ps = attnT.T @ Wproj8 (DR); x1 = srcw + mq/sp*ps (DVE stt, in place)
  LN1 -> x1n bf16 (ts, tt*g1, tt+beta1) -> PE-transpose -> x1nT fp8
  FFN1: z = W18 @ x1nT (DR); zT = gelu(z/s1 + b1) -> fp8 (ACT)
  FFN2: y = zT.T @ W28 (DR) + b2*s2 (ones matmul);
        out = y/s2 + x1n (DVE stt) -> DMA
"""

import numpy as np
import ml_dtypes

import concourse.bass as bass
import concourse.tile as tile
from concourse import bacc
from concourse import mybir
from concourse.bass_utils import run_bass_kernel_spmd

B, N, D, H, F = 8, 1024, 1024, 16, 4096
HD = D // H          # 64
P = 128
FC = D // P          # 8 feature chunks of D
TT = N // P          # 8 token tiles
GC = F // P          # 32 chunks of F
NEG = -1e30
EPS = 1e-5

f32 = mybir.dt.float32
bf16 = mybir.dt.bfloat16
fp8 = mybir.dt.float8e4
AF = mybir.ActivationFunctionType
OP = mybir.AluOpType
DR = mybir.MatmulPerfMode.DoubleRow


def _ln_stats(nc, pools, x_ap):
    """(mean, rstd) APs ([128,1]) for x_ap [128, 1024] f32."""
    stats = pools["stats"].tile([P, 2, 6], f32)
    for sg in range(2):
        nc.vector.bn_stats(out=stats[:, sg, :], in_=x_ap[:, sg * 512:(sg + 1) * 512])
    mv = pools["mv"].tile([P, 2], f32)
    nc.vector.bn_aggr(out=mv[:], in_=stats[:])
    nc.scalar.activation(out=mv[:, 1:2], in_=mv[:, 1:2], func=AF.Sqrt,
                         bias=pools["eps"][:], scale=1.0)
    nc.vector.reciprocal(out=mv[:, 1:2], in_=mv[:, 1:2])
    return mv[:, 0:1], mv[:, 1:2]


def build_bass(sw, sp, s1, s2):
    nc = bacc.Bacc("TRN2")

    src_h = nc.dram_tensor("src", [N, D], f32, kind="ExternalInput")
    kb_h = nc.dram_tensor("kbias", [TT, P], f32, kind="ExternalInput")
    mq_h = nc.dram_tensor("mq", [TT, P], f32, kind="ExternalInput")
    vf_h = nc.dram_tensor("vecsf", [1, D], f32, kind="ExternalInput")   # bproj
    # g1, beta1, b2*s2, bproj*sp
    vb_h = nc.dram_tensor("vecsb", [4, D], bf16, kind="ExternalInput")
    g0r_h = nc.dram_tensor("g0r", [FC, P], f32, kind="ExternalInput")
    beta0r_h = nc.dram_tensor("beta0r", [FC, P], f32, kind="ExternalInput")
    b1_h = nc.dram_tensor("b1r", [GC, P], f32, kind="ExternalInput")
    wqkv_h = nc.dram_tensor("wqkvT", [FC, P, 3 * D], fp8, kind="ExternalInput")
    wproj_h = nc.dram_tensor("wprojT", [FC, P, D], fp8, kind="ExternalInput")
    w1_h = nc.dram_tensor("w1T", [FC, P, F], fp8, kind="ExternalInput")
    w2_h = nc.dram_tensor("w2T", [GC, P, D], fp8, kind="ExternalInput")
    w2lo_h = nc.dram_tensor("w2loT", [GC, P, D], fp8, kind="ExternalInput")
    out_h = nc.dram_tensor("out", [N, D], f32, kind="ExternalOutput")

    with TileKernel(nc) as tk:
        tk.run(sw, sp, s1, s2, src_h, kb_h, mq_h, vf_h, vb_h, g0r_h,
               beta0r_h, b1_h, wqkv_h, wproj_h, w1_h, w2_h, w2lo_h, out_h)
    nc.compile()
    return nc


class TileKernel:
    def __init__(self, nc):
        self.nc = nc
        self.tc = tile.TileContext(nc)

    def __enter__(self):
        self.tc.__enter__()
        return self

    def __exit__(self, *a):
        return self.tc.__exit__(*a)

    def run(self, sw, sp, s1, s2, src_h, kb_h, mq_h, vf_h, vb_h, g0r_h,
            beta0r_h, b1_h, wqkv_h, wproj_h, w1_h, w2_h, w2lo_h, out_h):
        nc, tc = self.nc, self.tc
        from contextlib import ExitStack

        with ExitStack() as top:
            consts = top.enter_context(tc.tile_pool(name="consts", bufs=1))
            stats_pool = top.enter_context(tc.tile_pool(name="stats", bufs=3))
            mv_pool = top.enter_context(tc.tile_pool(name="mv", bufs=4))
            srcp = top.enter_context(tc.tile_pool(name="srcp", bufs=1))
            w1p = top.enter_context(tc.tile_pool(name="w1p", bufs=1))
            x1p = top.enter_context(tc.tile_pool(name="x1n", bufs=1))
            xtp = top.enter_context(tc.tile_pool(name="x1nT", bufs=1))
            cpool = tc.alloc_tile_pool(name="cpool", bufs=1)

            # ---------- constants ----------
            ident = consts.tile([P, P], bf16)
            from concourse.masks import make_identity
            make_identity(nc, ident[:])
            ident32 = consts.tile([P, P], f32)
            make_identity(nc, ident32[:])
            ones_row_bf = consts.tile([1, P], bf16)
            nc.vector.memset(ones_row_bf[:], 1.0)
            ones_col8 = consts.tile([P, 2, 16], fp8)
            nc.vector.memset(ones_col8[:], 1.0)
            eps_sb = consts.tile([P, 1], f32)
            nc.vector.memset(eps_sb[:], EPS)
            pools = {"stats": stats_pool, "mv": mv_pool, "eps": eps_sb}

            # broadcast vectors
            bcb = cpool.tile([P, 2, D], bf16)   # g1, beta1 bf16 bcast

            def _bcast_bf(i):
                ap = bass.AP(tensor=vb_h[0:1, :].tensor, offset=i * D,
                             ap=[[0, P], [1, D]])
                nc.sync.dma_start(out=bcb[:, i, :], in_=ap)

            src_sb = srcp.tile([P, TT, D], f32)   # src -> srcw -> x1 (in place)
            for tt in range(TT):
                nc.sync.dma_start(out=src_sb[:, tt, :],
                                  in_=src_h[tt * P:(tt + 1) * P, :])
            g0T_sb = consts.tile([P, FC], f32)
            nc.sync.dma_start(out=g0T_sb[:], in_=g0r_h[:, :].rearrange("a p -> p a"))
            beta0T_sb = consts.tile([P, FC], f32)
            nc.sync.dma_start(out=beta0T_sb[:],
                              in_=beta0r_h[:, :].rearrange("a p -> p a"))
            kb_sb = consts.tile([P, TT], f32)
            nc.sync.dma_start(out=kb_sb[:], in_=kb_h[:, :].rearrange("a p -> p a"))
            mq_sb = consts.tile([P, TT], f32)
            nc.sync.dma_start(out=mq_sb[:], in_=mq_h[:, :].rearrange("a p -> p a"))
            bprojb = consts.tile([P, D], f32)
            nc.sync.dma_start(out=bprojb[:],
                              in_=bass.AP(tensor=vf_h[0:1, :].tensor, offset=0,
                                          ap=[[0, P], [1, D]]))
            b2s2_sb = consts.tile([1, D], bf16)
            nc.sync.dma_start(out=b2s2_sb[:], in_=vb_h[2:3, :])
            bprojsp_sb = consts.tile([1, D], bf16)
            nc.sync.dma_start(out=bprojsp_sb[:], in_=vb_h[3:4, :])
            _bcast_bf(0)
            _bcast_bf(1)
            invmq_sb = consts.tile([P, TT], f32)
            nc.vector.tensor_scalar(out=invmq_sb[:], in0=mq_sb[:], scalar1=-1.0,
                                    scalar2=1.0, op0=OP.mult, op1=OP.add)
            mqsp_sb = consts.tile([P, TT], f32)
            nc.vector.tensor_scalar(out=mqsp_sb[:], in0=mq_sb[:],
                                    scalar1=1.0 / sp, scalar2=None, op0=OP.mult)
            b1_sb = consts.tile([P, GC], f32)
            nc.sync.dma_start(out=b1_sb[:], in_=b1_h[:, :].rearrange("g p -> p g"))

            g1b, beta1b = bcb[:, 0], bcb[:, 1]

            wb_sb = cpool.tile([P, D], f32)     # masked-query blend vector
            u_sb = consts.tile([P, FC, 16], fp8)  # mean_j v (col 0), 16B stride

            with ExitStack() as attn_scope:
                qkp = attn_scope.enter_context(tc.tile_pool(name="qkT", bufs=1))
                vp = attn_scope.enter_context(tc.tile_pool(name="vsb", bufs=1))
                wqkp = attn_scope.enter_context(tc.tile_pool(name="wqk", bufs=1))
                atp = attn_scope.enter_context(tc.tile_pool(name="attnT", bufs=1))
                wpp = attn_scope.enter_context(tc.tile_pool(name="wproj", bufs=1))

                qkT_bf = qkp.tile([P, 16, N], bf16)
                v_sb = vp.tile([P, TT // 2, 2, H, HD + 1], fp8)
                nc.vector.memset(v_sb[:, :, :, :, HD:HD + 1], 1.0)
                attnT_sb = atp.tile([P, FC, N], fp8)
                wproj_sb = wpp.tile([P, FC, D], fp8)

                # ================= LN0 + transpose =================
                with ExitStack() as qkv_scope:
                    htp = qkv_scope.enter_context(tc.tile_pool(name="hT", bufs=1))
                    hT_sb = htp.tile([P, FC, N], fp8)

                    with tc.tile_pool(name="hbf", bufs=2) as hbp, \
                            tc.tile_pool(name="trps", bufs=4, space="PSUM") as trps:
                        for tt in range(TT):
                            x = src_sb[:, tt, :]
                            mean, rstd = _ln_stats(nc, pools, x)
                            # nmr = -(mean*rstd) so ACT computes (x-m)*r as
                            # Identity(x*rstd + nmr)
                            nmr = pools["mv"].tile([P, 1], f32, tag="nmr")
                            nc.vector.tensor_scalar(out=nmr[:], in0=mean,
                                                    scalar1=rstd, scalar2=-1.0,
                                                    op0=OP.mult, op1=OP.mult)
                            hbf = hbp.tile([P, D], bf16)
                            nc.scalar.activation(out=hbf[:], in_=x,
                                                 func=AF.Identity,
                                                 bias=nmr[:], scale=rstd)
                            for q4 in range(4):
                                ps = trps.tile([P, 2, P], bf16)
                                for s in range(2):
                                    fb = q4 * 2 + s
                                    nc.tensor.transpose(
                                        ps[:, s, :], hbf[:, fb * P:(fb + 1) * P],
                                        ident[:])
                                # copies fold h = h'*g0 + beta0 (feature-major
                                # per-partition scalars), split across engines
                                for s in range(2):
                                    fb = q4 * 2 + s
                                    dst = hT_sb[:, fb, tt * P:(tt + 1) * P]
                                    if (fb + tt) % 2 == 0:
                                        nc.vector.tensor_scalar(
                                            out=dst, in0=ps[:, s, :],
                                            scalar1=g0T_sb[:, fb:fb + 1],
                                            scalar2=beta0T_sb[:, fb:fb + 1],
                                            op0=OP.mult, op1=OP.add)
                                    else:
                                        nc.scalar.activation(
                                            out=dst, in_=ps[:, s, :],
                                            func=AF.Identity,
                                            bias=beta0T_sb[:, fb:fb + 1],
                                            scale=g0T_sb[:, fb:fb + 1])

                    # ---- weight DMAs for v/qk/proj ----
                    wvp = qkv_scope.enter_context(tc.tile_pool(name="wv", bufs=1))
                    wv_sb = wvp.tile([P, FC, D], fp8)
                    nc.sync.dma_start(
                        out=wv_sb[:],
                        in_=wqkv_h[:, :, 2 * D:3 * D].rearrange("f p o -> p f o"))
                    wqk_sb = wqkp.tile([P, FC, 2 * D], fp8)
                    nc.sync.dma_start(
                        out=wqk_sb[:],
                        in_=wqkv_h[:, :, 0:2 * D].rearrange("f p o -> p f o"))
                    nc.sync.dma_start(
                        out=wproj_sb[:],
                        in_=wproj_h[:, :, :].rearrange("f p o -> p f o"))

                    # ---- interleaved qk/v + attention per head-pair ----
                    # v matmuls are spread into the h0/h1 exp window (PE is
                    # otherwise idle there); AV for h0/h1 is deferred until v
                    # lands. qk and v share one psum pool (bank budget).
                    with tc.tile_pool(name="bigps", bufs=1, space="PSUM") as bigps, \
                            tc.tile_pool(name="sps", bufs=2, space="PSUM") as sps, \
                            tc.tile_pool(name="avps", bufs=1, space="PSUM") as avps, \
                            tc.tile_pool(name="bcps", bufs=1, space="PSUM") as bcps, \
                            tc.tile_pool(name="pt", bufs=12) as ptp, \
                            tc.tile_pool(name="rd", bufs=3) as rdp:
                        def emit_v(tts):
                            for tt in tts:
                                ps = bigps.tile([P, N], f32)
                                for vb in range(2):
                                    for fp_ in range(4):
                                        nc.tensor.matmul(
                                            ps[:, vb * 512:(vb + 1) * 512],
                                            hT_sb[:, 2 * fp_:2 * fp_ + 2,
                                                  tt * P:(tt + 1) * P],
                                            wv_sb[:, 2 * fp_:2 * fp_ + 2,
                                                  vb * 512:(vb + 1) * 512],
                                            start=(fp_ == 0), stop=(fp_ == 3),
                                            perf_mode=DR)
                                nc.vector.tensor_scalar(
                                    out=v_sb[:, tt // 2, tt % 2, :, 0:HD],
                                    in0=ps[:].rearrange("p (h c) -> p h c", h=H),
                                    scalar1=1.0 / sw, scalar2=None, op0=OP.mult)

                        def emit_S_exp(h):
                            hq = (h % 2) * HD
                            fc_h = h // 2
                            pts = []
                            for jc in range(TT):
                                ps_s = sps.tile([P, N], f32)
                                for ib in range(2):
                                    nc.tensor.matmul(
                                        ps_s[:, ib * 512:(ib + 1) * 512],
                                        qkT_bf[hq:hq + HD, 8 + fc_h,
                                               jc * P:(jc + 1) * P],
                                        qkT_bf[hq:hq + HD, fc_h,
                                               ib * 512:(ib + 1) * 512],
                                        start=True, stop=True)
                                if jc % 2 == 0:
                                    pt = ptp.tile([P, 2, N], fp8)
                                    pts.append(pt)
                                nc.scalar.activation(
                                    out=pts[jc // 2][:, jc % 2, :], in_=ps_s[:],
                                    func=AF.Exp, bias=kb_sb[:, jc:jc + 1],
                                    scale=0.125)
                            return pts

                        def emit_AV(h, pts):
                            hq = (h % 2) * HD
                            fc_h = h // 2
                            for ib in range(2):
                                isl = slice(ib * 512, (ib + 1) * 512)
                                ps_av = avps.tile([P, 512], f32)
                                for jp in range(4):
                                    nc.tensor.matmul(
                                        ps_av[0:HD + 1, :],
                                        v_sb[:, jp, :, h, :],
                                        pts[jp][:, :, isl],
                                        start=(jp == 0), stop=(jp == 3),
                                        perf_mode=DR)
                                rd = rdp.tile([1, 512], bf16)
                                with nc.allow_low_precision(
                                        reason="softmax denom recip; "
                                        "errors cancel via dilution"):
                                    nc.vector.reciprocal(
                                        rd[:], ps_av[HD:HD + 1, :])
                                ps_b = bcps.tile([P, 512], f32)
                                nc.tensor.matmul(ps_b[0:HD, :],
                                                 ones_row_bf[:, 0:HD],
                                                 rd[:], start=True, stop=True)
                                rb = rdp.tile([HD, 512], bf16, tag="rb")
                                nc.vector.tensor_copy(rb[:], ps_b[0:HD, :])
                                nc.vector.tensor_tensor(
                                    attnT_sb[hq:hq + HD, fc_h, isl],
                                    ps_av[0:HD, :], rb[:], OP.mult)

                        w1_sb = w1p.tile([P, FC, F], fp8)
                        held = {}
                        for hp in range(8):
                            if hp == 2:
                                for wh in range(2):
                                    nc.sync.dma_start(
                                        out=w1_sb[:, :, wh * 2048:(wh + 1) * 2048],
                                        in_=w1_h[:, :, wh * 2048:(wh + 1) * 2048]
                                        .rearrange("f p o -> p f o"))
                            for oc in (hp, 8 + hp):  # q chunk, k chunk
                                ps = bigps.tile([P, N], f32)
                                for ib in range(2):
                                    for fp_ in range(4):
                                        nc.tensor.matmul(
                                            ps[:, ib * 512:(ib + 1) * 512],
                                            wqk_sb[:, 2 * fp_:2 * fp_ + 2,
                                                   oc * P:(oc + 1) * P],
                                            hT_sb[:, 2 * fp_:2 * fp_ + 2,
                                                  ib * 512:(ib + 1) * 512],
                                            start=(fp_ == 0), stop=(fp_ == 3),
                                            perf_mode=DR)
                                if oc < 8:
                                    nc.vector.tensor_scalar(
                                        out=qkT_bf[:, oc, :], in0=ps[:],
                                        scalar1=1.0 / sw, scalar2=None,
                                        op0=OP.mult)
                                elif hp < 2:
                                    nc.scalar.activation(
                                        out=qkT_bf[:, oc, :], in_=ps[:],
                                        func=AF.Copy, bias=0.0, scale=1.0 / sw)
                                else:
                                    nc.scalar.activation(
                                        out=qkT_bf[:, oc, :], in_=ps[:],
                                        func=AF.Copy, bias=0.0, scale=1.0 / sw)

                            if hp == 1:   # v landed; catch up on h0/h1 AV
                                for h0 in (0, 1):
                                    emit_AV(h0, held.pop(h0))
                            for h in (2 * hp, 2 * hp + 1):
                                pts = emit_S_exp(h)
                                if hp == 0:
                                    held[h] = pts
                                    emit_v(range(0, 4) if h == 0 else range(4, 8))
                                else:
                                    emit_AV(h, pts)

                # ---- u = mean_j v; wb = bcast(Wproj @ u / sp + bproj) ----
                with tc.tile_pool(name="uwps", bufs=1, space="PSUM") as uwps, \
                        tc.tile_pool(name="urowp", bufs=1) as urowp:
                    urow_ps = uwps.tile([1, H, HD], f32, tag="urow")
                    for h in range(H):
                        for jp in range(TT // 2):
                            nc.tensor.matmul(
                                urow_ps[0:1, h, :], ones_col8[:, :, 0:1],
                                v_sb[:, jp, :, h, 0:HD],
                                start=(jp == 0), stop=(jp == TT // 2 - 1),
                                perf_mode=DR)
                    urow_sb = urowp.tile([1, H, HD], bf16)
                    nc.vector.tensor_scalar(out=urow_sb[:], in0=urow_ps[:],
                                            scalar1=1.0 / N, scalar2=None,
                                            op0=OP.mult)
                    u2ps = uwps.tile([P, FC], f32, tag="u2")
                    for fc in range(FC):
                        nc.tensor.matmul(
                            u2ps[:, fc:fc + 1],
                            urow_sb[0:1, 2 * fc:2 * fc + 2, :],
                            ones_row_bf[0:1, 0:1],
                            start=True, stop=True)
                    nc.vector.tensor_copy(u_sb[:, :, 0], u2ps[:])
                    wrow = urowp.tile([1, D], bf16, tag="wrow")
                    for ob in range(2):
                        osl = slice(ob * 512, (ob + 1) * 512)
                        ps = uwps.tile([P, 512], f32, tag="wps")
                        for fp_ in range(FC // 2):
                            nc.tensor.matmul(ps[0:1, :],
                                             u_sb[:, 2 * fp_:2 * fp_ + 2, 0:1],
                                             wproj_sb[:, 2 * fp_:2 * fp_ + 2, osl],
                                             start=(fp_ == 0),
                                             stop=(fp_ == FC // 2 - 1),
                                             perf_mode=DR)
                        nc.vector.scalar_tensor_tensor(
                            out=wrow[:, osl], in0=ps[0:1, :], scalar=1.0 / sp,
                            in1=bprojb[0:1, osl], op0=OP.mult, op1=OP.add)
                    for ob in range(2):
                        osl = slice(ob * 512, (ob + 1) * 512)
                        ps = uwps.tile([P, 512], f32, tag="wbps")
                        nc.tensor.matmul(ps[:], ones_row_bf[:], wrow[:, osl],
                                         start=True, stop=True)
                        nc.vector.tensor_copy(wb_sb[:, osl], ps[:])

                # srcw = src + (1-mq)*wb, split DVE/Pool halves
                for tt in range(TT):
                    for ob in range(2):
                        osl = slice(ob * 512, (ob + 1) * 512)
                        nc.vector.scalar_tensor_tensor(
                            out=src_sb[:, tt, osl], in0=wb_sb[:, osl],
                            scalar=invmq_sb[:, tt:tt + 1],
                            in1=src_sb[:, tt, osl],
                            op0=OP.mult, op1=OP.add)

                # ====== proj + x1 (in place) + LN1 + transpose, per tt ======
                x1n_sb = x1p.tile([P, TT, D], bf16)
                x1nT_sb = xtp.tile([P, FC, N], fp8)

                with tc.tile_pool(name="pps", bufs=2, space="PSUM") as pps, \
                        tc.tile_pool(name="trps2", bufs=2, space="PSUM") as trps2:
                    for tt in range(TT):
                        for ob in range(2):
                            osl = slice(ob * 512, (ob + 1) * 512)
                            ps_p = pps.tile([P, 512], f32)
                            nc.tensor.matmul(ps_p[:], ones_row_bf[:],
                                             bprojsp_sb[0:1, osl],
                                             start=True, stop=False)
                            for fp_ in range(4):
                                nc.tensor.matmul(
                                    ps_p[:],
                                    attnT_sb[:, 2 * fp_:2 * fp_ + 2,
                                             tt * P:(tt + 1) * P],
                                    wproj_sb[:, 2 * fp_:2 * fp_ + 2, osl],
                                    start=False, stop=(fp_ == 3),
                                    perf_mode=DR)
                            nc.vector.scalar_tensor_tensor(
                                out=src_sb[:, tt, osl], in0=ps_p[:],
                                scalar=mqsp_sb[:, tt:tt + 1],
                                in1=src_sb[:, tt, osl], op0=OP.mult, op1=OP.add)

                        x = src_sb[:, tt, :]
                        mean, rstd = _ln_stats(nc, pools, x)
                        nmr = pools["mv"].tile([P, 1], f32, tag="nmr")
                        nc.vector.tensor_scalar(out=nmr[:], in0=mean,
                                                scalar1=rstd, scalar2=-1.0,
                                                op0=OP.mult, op1=OP.mult)
                        xbn = x1n_sb[:, tt, :]
                        nc.scalar.activation(out=xbn, in_=x, func=AF.Identity,
                                             bias=nmr[:], scale=rstd)
                        teng = nc.vector if tt % 2 == 0 else nc.gpsimd
                        teng.tensor_tensor(xbn, xbn, g1b, OP.mult)
                        nc.vector.tensor_tensor(xbn, xbn, beta1b, OP.add)
                        for q4 in range(4):
                            ps = trps2.tile([P, 2, P], bf16)
                            for s in range(2):
                                fb = q4 * 2 + s
                                nc.tensor.transpose(
                                    ps[:, s, :], xbn[:, fb * P:(fb + 1) * P], ident[:])
                            dst = x1nT_sb[:, q4 * 2:q4 * 2 + 2,
                                          tt * P:(tt + 1) * P]
                            if q4 % 2 == 0:
                                nc.scalar.copy(dst, ps[:])
                            else:
                                nc.vector.tensor_copy(dst, ps[:])

            cpool.release()

            # ================= FFN =================
            # FFN2 runs in two stages so the last gelus don't gate all of it:
            # stage 1 (gp 0..7) overlaps the second half of the gelu stream,
            # parking partial sums in src_sb (x1 is dead after LN1); stage 2
            # reloads them into psum via an identity matmul (f32r read).
            with ExitStack() as ffn_scope:
                ztp = ffn_scope.enter_context(tc.tile_pool(name="zT", bufs=1))
                zT_sb = ztp.tile([P, GC, N], fp8)     # gelu out, fp8 hi part
                zlo_sb = ztp.tile([P, GC, N], fp8)    # fp8 residual (lo part)
                gbp = ffn_scope.enter_context(tc.tile_pool(name="gtmp", bufs=2))
                w2p = ffn_scope.enter_context(tc.tile_pool(name="w2p", bufs=1))

                with tc.tile_pool(name="zps", bufs=2, space="PSUM") as zps, \
                        tc.tile_pool(name="yout", bufs=4) as yout, \
                        tc.tile_pool(name="yps", bufs=2, space="PSUM") as yps:
                    for gc in range(GC):
                        ps = zps.tile([P, N], f32)
                        for ib in range(2):
                            for fp_ in range(4):
                                nc.tensor.matmul(
                                    ps[:, ib * 512:(ib + 1) * 512],
                                    w1_sb[:, 2 * fp_:2 * fp_ + 2,
                                          gc * P:(gc + 1) * P],
                                    x1nT_sb[:, 2 * fp_:2 * fp_ + 2,
                                            ib * 512:(ib + 1) * 512],
                                    start=(fp_ == 0), stop=(fp_ == 3),
                                    perf_mode=DR)
                        gtmp = gbp.tile([P, N], bf16)
                        nc.scalar.activation(out=gtmp[:], in_=ps[:],
                                             func=AF.Gelu,
                                             bias=b1_sb[:, gc:gc + 1],
                                             scale=1.0 / s1)
                        nc.vector.tensor_copy(zT_sb[:, gc, :], gtmp[:])
                        nc.vector.scalar_tensor_tensor(
                            out=zlo_sb[:, gc, :], in0=gtmp[:], scalar=1.0,
                            in1=zT_sb[:, gc, :], op0=OP.mult, op1=OP.subtract)
                        if gc == 15:
                            # ---- FFN2 stage 1 (ob 0): 3-pass over gp 0..7 ----
                            w2q0 = w2p.tile([P, 2, GC, 512], fp8, tag="w2q")
                            nc.sync.dma_start(
                                out=w2q0[:, 0], in_=w2_h[:, :, 0:512]
                                .rearrange("g p c -> p g c"))
                            nc.sync.dma_start(
                                out=w2q0[:, 1], in_=w2lo_h[:, :, 0:512]
                                .rearrange("g p c -> p g c"))
                            for tt in range(TT):
                                psy = yps.tile([P, 512], f32)
                                for i, (zt, wi) in enumerate(
                                        ((zT_sb, 0), (zlo_sb, 0), (zT_sb, 1))):
                                    for gp in range(8):
                                        nc.tensor.matmul(
                                            psy[:],
                                            zt[:, 2 * gp:2 * gp + 2,
                                               tt * P:(tt + 1) * P],
                                            w2q0[:, wi, 2 * gp:2 * gp + 2, :],
                                            start=(i == 0 and gp == 0),
                                            stop=(i == 2 and gp == 7),
                                            perf_mode=DR)
                                nc.vector.tensor_copy(src_sb[:, tt, 0:512],
                                                      psy[:])

                    # ---- FFN2 remaining: ob0 gp 8..15, then full ob1 ----
                    for ob in range(2):
                        osl = slice(ob * 512, (ob + 1) * 512)
                        if ob == 1:
                            w2q = w2p.tile([P, 2, GC, 512], fp8, tag="w2q")
                            nc.sync.dma_start(
                                out=w2q[:, 0], in_=w2_h[:, :, osl]
                                .rearrange("g p c -> p g c"))
                            nc.sync.dma_start(
                                out=w2q[:, 1], in_=w2lo_h[:, :, osl]
                                .rearrange("g p c -> p g c"))
                        else:
                            w2q = w2q0
                        for tt in range(TT):
                            psy = yps.tile([P, 512], f32)
                            nc.tensor.matmul(psy[:], ones_row_bf[:],
                                             b2s2_sb[0:1, osl],
                                             start=True, stop=False)
                            if ob == 0:
                                nc.tensor.matmul(
                                    psy[:], ident32[:].bitcast(mybir.dt.float32r),
                                    src_sb[:, tt, osl].bitcast(mybir.dt.float32r),
                                    start=False, stop=False)
                            gplo = 8 if ob == 0 else 0
                            for i, (zt, wi) in enumerate(
                                    ((zT_sb, 0), (zlo_sb, 0), (zT_sb, 1))):
                                for gp in range(gplo, 16):
                                    nc.tensor.matmul(
                                        psy[:],
                                        zt[:, 2 * gp:2 * gp + 2,
                                           tt * P:(tt + 1) * P],
                                        w2q[:, wi, 2 * gp:2 * gp + 2, :],
                                        start=False,
                                        stop=(i == 2 and gp == 15),
                                        perf_mode=DR)
                            t = yout.tile([P, 512], f32)
                            nc.vector.scalar_tensor_tensor(
                                out=t[:], in0=psy[:], scalar=1.0 / s2,
                                in1=x1n_sb[:, tt, osl], op0=OP.mult, op1=OP.add)
                            nc.sync.dma_start(out=out_h[tt * P:(tt + 1) * P, osl],
                                              in_=t[:])

_NC_CACHE = {}


def _get_nc(scales=(1024.0, 1024.0, 1024.0, 1024.0)):
    key = tuple(float(s) for s in scales)
    if key not in _NC_CACHE:
        _NC_CACHE[key] = build_bass(*key)
    return _NC_CACHE[key]


def _pow2_scale(w):
    a = float(np.abs(w).max())
    if a == 0.0:
        return 1.0
    return float(2.0 ** np.floor(np.log2(224.0 / a)))


def prep_in_maps(inputs):
    src = np.asarray(inputs["src"], dtype=np.float32)          # [B, N, D]
    mask = np.asarray(inputs["mask"])                          # [B, N] bool
    Wqkv = np.asarray(inputs["Wqkv"], dtype=np.float32)
    Wproj = np.asarray(inputs["Wproj"], dtype=np.float32)
    bproj = np.asarray(inputs["bproj"], dtype=np.float32)
    W1 = np.asarray(inputs["W1"], dtype=np.float32)
    b1 = np.asarray(inputs["b1"], dtype=np.float32)
    W2 = np.asarray(inputs["W2"], dtype=np.float32)
    b2 = np.asarray(inputs["b2"], dtype=np.float32)
    g0 = np.asarray(inputs["g0"], dtype=np.float32)
    beta0 = np.asarray(inputs["beta0"], dtype=np.float32)
    g1 = np.asarray(inputs["g1"], dtype=np.float32)
    beta1 = np.asarray(inputs["beta1"], dtype=np.float32)

    sw = _pow2_scale(Wqkv)
    sp = _pow2_scale(Wproj)
    s1 = _pow2_scale(W1)
    s2 = _pow2_scale(W2)

    e4 = ml_dtypes.float8_e4m3
    bf = ml_dtypes.bfloat16
    wqkvT = np.ascontiguousarray((Wqkv * sw).T).reshape(FC, P, 3 * D).astype(e4)
    wprojT = np.ascontiguousarray((Wproj * sp).T).reshape(FC, P, D).astype(e4)
    w1T = np.ascontiguousarray((W1 * s1).T).reshape(FC, P, F).astype(e4)
    w2s = np.ascontiguousarray((W2 * s2).T).reshape(GC, P, D)
    w2T = w2s.astype(e4)
    w2loT = (w2s - w2T.astype(np.float32)).astype(e4)
    vecsf = np.ascontiguousarray(bproj).reshape(1, D)
    vecsb = np.ascontiguousarray(
        np.stack([g1, beta1, b2 * s2, bproj * sp])).astype(bf)
    g0r = np.ascontiguousarray(g0.reshape(FC, P))
    beta0r = np.ascontiguousarray(beta0.reshape(FC, P))
    b1r = np.ascontiguousarray(b1.reshape(GC, P))
    kbias = np.where(mask, 0.0, NEG).astype(np.float32).reshape(B, TT, P)
    mqf = mask.astype(np.float32).reshape(B, TT, P)

    in_maps = []
    for b in range(B):
        in_maps.append({
            "src": np.ascontiguousarray(src[b]),
            "kbias": np.ascontiguousarray(kbias[b]),
            "mq": np.ascontiguousarray(mqf[b]),
            "vecsf": vecsf,
            "vecsb": vecsb,
            "g0r": g0r,
            "beta0r": beta0r,
            "b1r": b1r,
            "wqkvT": wqkvT,
            "wprojT": wprojT,
            "w1T": w1T,
            "w2T": w2T,
            "w2loT": w2loT,
        })
    return in_maps, (sw, sp, s1, s2)


def kernel(**inputs):
    in_maps, scales = prep_in_maps(inputs)
    nc = _get_nc(scales)
    res = run_bass_kernel_spmd(nc, in_maps, core_ids=list(range(B)))
    return np.stack([r["out"] for r in res.results]).astype(np.float32)
